# revision 21
# baseline (speedup 1.0000x reference)
"""Causal single-head attention (B=4, S=2048, D=1024) on 8 NeuronCores.

Sharding: core c owns the q rows {2i + (c%2)} of batch c//2 (1024 rows).
Interleaving q rows by parity gives every core an identical causal
block structure, so one SPMD program serves all 8 cores; only the data
(and the staircase mask) differs per core.

Key order is globally redefined as [parity-0 rows asc, parity-1 rows
asc] — attention is invariant to key permutation as long as K, V and
the mask agree. Under that order each core's q rows are its own parity
half, its causal extent per q-block j is the uniform tile set
[0, 4(j+1)) + [8, 8+4(j+1)) (128-key tiles), and exactly 8 tiles per
block cross the diagonal. Crossing tile with in-block offset c is
fully masked on its first 128*c q columns: scores/exp run only on the
remaining columns and AV matmuls for q-subtiles u < c are skipped.

K/V projections are deduplicated across the core pair of each batch:
core p computes K/V only for its parity rows, and the pair exchanges
halves with 2-core AllGathers (DRAM bounce).

Scheduling (v2):
- Inputs are host-packed in per-d pairs (xo|wk and wv|wq share a DMA)
  and triggered alternately from the Sync and Scalar queues, so the
  trigger-issue serialization (~0.65us each) halves and the first
  d-tile lands ~3us after the preamble. wv/wq/mask are deferred behind
  early stage-A matmuls so the ramp-critical xo+wk stream gets full
  DMA bandwidth.
- Stages A (kT), B (v), C (qT) each run as 2 waves of 8 concurrent
  PSUM groups (all 8 banks), d-outermost, so during the input ramp the
  PE advances as each d pair lands.
- The 4 pair-AllGathers (k x2, v x2) are triggered on gpsimd the
  moment their wave's staging DMA completes; the DRAM->SBUF read-backs
  are emitted AFTER all sends, so the Sync FIFO never head-of-line
  blocks a later exchange behind an earlier collective's completion
  (the baseline lost ~10us to that daisy-chain).
- Softmax denominators ride the AV loop as N=1 matmuls reusing the AV
  stationary operand.
- The output is written bf16 (host upcasts); halves the tail drain.
"""

import sys
import types

import numpy as np
import ml_dtypes

import concourse.tile as tile
from concourse import bacc, mybir
from concourse.bass_utils import run_bass_kernel_spmd


def _ensure_ntff_hook():
    """bass_utils imports antenv.axon_hooks when tracing; some containers
    lack that module. Register a process-local equivalent so trace=True
    works (or degrades to untraced instead of crashing)."""
    try:
        import antenv.axon_hooks  # noqa: F401
        return
    except ImportError:
        pass
    hook = None
    try:
        from trn_agent_boot.trn_boot import _ntff_profile_via_ctypes
        hook = _ntff_profile_via_ctypes("/opt/axon/libaxon_pjrt.so")
    except Exception:
        hook = None
    mod = types.ModuleType("antenv.axon_hooks")
    mod.get_axon_ntff_profile_hook = lambda: hook
    mod.set_axon_ntff_profile_hook = lambda h: None
    sys.modules["antenv.axon_hooks"] = mod


_ensure_ntff_hook()

BF16 = mybir.dt.bfloat16
F32 = mybir.dt.float32
AF = mybir.ActivationFunctionType

B, S, D = 4, 2048, 1024
P = 128
NCORES = 8
SQ = 1024            # q rows per core (= own parity half)
ND = D // P          # 8 contraction tiles over d
NE = D // P          # 8 tiles over e (d_out)
NSK = S // P         # 16 key tiles
QB = 512             # q-block width (matmul free dim)
NQB = SQ // QB       # 2 q blocks
SCALE = 1.0 / np.sqrt(np.float32(D))
PAIRS = [[2 * b, 2 * b + 1] for b in range(B)]

TRACE = False
LAST_RESULT = None

_cache = {}


def _sk_list(j):
    # key tiles needed by q-block j: prefix of each parity half
    return list(range(0, 4 * (j + 1))) + list(range(8, 8 + 4 * (j + 1)))


def _cross_list(j):
    # diagonal-crossing key tiles of q-block j (order matches maskd[j])
    return list(range(4 * j, 4 * (j + 1))) + list(range(8 + 4 * j, 8 + 4 * (j + 1)))


def _coff(j, t):
    # in-block crossing offset: first 128*c q columns of tile t are fully
    # masked within q-block j (c = 0 for non-crossing computed tiles)
    return max(0, (t % 8) - 4 * j)


def _build():
    nc = bacc.Bacc("TRN2", target_bir_lowering=False, debug=False,
                   num_devices=NCORES)
    # host-packed pairs: xkt[d] = [xo_d | wk_d], wvqt[d] = [wv_d | wq_d]
    xkt = nc.dram_tensor("xkt", [ND, P, 2 * SQ], BF16, kind="ExternalInput")
    wvqt = nc.dram_tensor("wvqt", [ND, P, 2 * D], BF16, kind="ExternalInput")
    maskd = nc.dram_tensor("maskd", [P, NQB, 8, QB], BF16, kind="ExternalInput")
    ones = nc.dram_tensor("ones", [P, 8], BF16, kind="ExternalInput")
    out = nc.dram_tensor("out", [SQ, D], BF16, kind="ExternalOutput")
    # pair-shared HBM scratchpad: cores (2k, 2k+1) see one physical buffer.
    # The K/V halves are exchanged by plain DMA through these (each core
    # cond-writes its parity slot, a tiny pair collective acts as the
    # barrier, then both cores read both slots back symmetrically) —
    # bypassing the slow CC data path entirely.
    ksh = nc.dram_tensor("ksh", [2, P, NE, SQ], BF16, addr_space="Shared")
    vsh = nc.dram_tensor("vsh", [2, P, 8, D], BF16, addr_space="Shared")

    from contextlib import ExitStack
    with tile.TileContext(nc) as tc:
        with ExitStack() as ctx:
            xk_pool = ctx.enter_context(tc.tile_pool(name="xk", bufs=1))
            wvq_pool = ctx.enter_context(tc.tile_pool(name="wvq", bufs=1))
            st_pool = ctx.enter_context(tc.tile_pool(name="st", bufs=2))
            kT_pool = ctx.enter_context(tc.tile_pool(name="kT", bufs=1))
            v_pool = ctx.enter_context(tc.tile_pool(name="v", bufs=1))
            qT_pool = ctx.enter_context(tc.tile_pool(name="qT", bufs=NE))
            m_pool = ctx.enter_context(tc.tile_pool(name="mk", bufs=1))
            we_pool = ctx.enter_context(tc.tile_pool(name="we", bufs=16))
            on_pool = ctx.enter_context(tc.tile_pool(name="on", bufs=1))
            wm_pool = ctx.enter_context(tc.tile_pool(name="wm", bufs=1))
            rc_pool = ctx.enter_context(tc.tile_pool(name="rc", bufs=4))
            o_pool = ctx.enter_context(tc.tile_pool(name="o", bufs=2))
            dr_pool = ctx.enter_context(
                tc.tile_pool(name="dr", bufs=4, space="DRAM"))
            # 8 PSUM banks: ps 2 + av 4 + rs 2 (A/B/C borrow all 8 as
            # [P,QB] wave accumulators; D uses them in their named roles)
            ps_pool = ctx.enter_context(
                tc.tile_pool(name="ps", bufs=2, space="PSUM"))
            av_pool = ctx.enter_context(
                tc.tile_pool(name="av", bufs=4, space="PSUM"))
            rs_pool = ctx.enter_context(
                tc.tile_pool(name="rs", bufs=2, space="PSUM"))

            from concourse.bass import _add_dep_helper

            # ---- input DMAs: xk pairs alternate sync/scalar queues,
            # chained two-at-a-time so the d=0 pair gets full bandwidth
            # (unchained, all 8 fair-share and d=0 lands ~4us later) ----
            xk_big = xk_pool.tile([P, ND, 2 * SQ], BF16, tag="xk")
            xk_dmas = []
            for d in range(ND):
                eng = nc.sync if d % 2 == 0 else nc.scalar
                dma = eng.dma_start(xk_big[:, d:d + 1, :],
                                    xkt[d:d + 1].rearrange("n p m -> p n m"))
                if d >= 2:
                    _add_dep_helper(dma.ins, xk_dmas[d - 2].ins, sync=True,
                                    reason="pace input stream behind ramp")
                xk_dmas.append(dma)
            xk = [xk_big[:, d, :] for d in range(ND)]
            # parity predicates for the shared-HBM exchange writes
            pid = nc.sync.partition_id()
            par1 = pid % 2
            par0 = (pid + 1) % 2
            # deferred bulk loads (gated below once the ramp-critical xk
            # stream is mostly in): wv/wq halves, mask, ones
            wvq_big = wvq_pool.tile([P, ND, 2 * D], BF16, tag="wvq")
            defer_a = [nc.scalar.dma_start(
                wvq_big[:, d:d + 4, :],
                wvqt[d:d + 4].rearrange("n p m -> p n m")) for d in (0, 4)]
            wvq = [wvq_big[:, d, :] for d in range(ND)]
            mask_big = m_pool.tile([P, NQB, 8, QB], BF16, tag="mk")
            ones_t = on_pool.tile([P, 8], BF16, tag="on")
            defer_b = [nc.scalar.dma_start(mask_big[:], maskd[:]),
                       nc.scalar.dma_start(ones_t[:], ones[:])]

            kT_big = kT_pool.tile([P, NE, S], BF16, tag="kT")
            v_big = v_pool.tile([P, NSK, D], BF16, tag="v")

            # PE clock (HAM) warmup during the initial DMA wait
            warm = wm_pool.tile([P, P], BF16, tag="warm")
            nc.vector.memset(warm[:], 0.0)
            wps = ps_pool.tile([P, P], F32, tag="ps")
            for i in range(24):
                nc.tensor.matmul(wps[:], warm[:], warm[:],
                                 start=(i == 0), stop=(i == 23))

            _wv = [0]

            def wave_psums():
                _wv[0] += 1
                tiles = []
                for g in range(8):
                    if g < 2:
                        pool, tg = ps_pool, "ps"
                    elif g < 6:
                        pool, tg = av_pool, "av"
                    else:
                        pool, tg = rs_pool, "rs"
                    tiles.append(pool.tile([P, QB], F32, tag=tg,
                                           name=f"wps{_wv[0]}g{g}"))
                return tiles

            # ---- stage A: kT own half [e, s0]; exchange per E-wave ----
            k_writes, v_writes = [], []
            for w in range(2):
                grp = [(E, Sc) for E in range(4 * w, 4 * w + 4)
                       for Sc in range(2)]
                pss = wave_psums()
                for d in range(ND):
                    for g, (E, Sc) in enumerate(grp):
                        mm = nc.tensor.matmul(
                            pss[g][:],
                            xk[d][:, SQ + E * P:SQ + (E + 1) * P],
                            xk[d][:, Sc * QB:(Sc + 1) * QB],
                            start=(d == 0), stop=(d == ND - 1),
                        )
                    if w == 0 and d == 4:
                        _add_dep_helper(defer_a[0].ins, mm.ins, sync=True,
                                        reason="defer wvq0 past ramp")
                    if w == 0 and d == 6:
                        _add_dep_helper(defer_a[1].ins, mm.ins, sync=True,
                                        reason="defer wvq1 past ramp")
                    if w == 1 and d == 2:
                        for dd in defer_b:
                            _add_dep_helper(dd.ins, mm.ins, sync=True,
                                            reason="defer mask past ramp")
                kst = st_pool.tile([P, 4, SQ], BF16, tag="st",
                                   name=f"kst{w}")
                for g, (E, Sc) in enumerate(grp):
                    nc.vector.tensor_copy(
                        kst[:, E - 4 * w, Sc * QB:(Sc + 1) * QB], pss[g][:])
                k_writes.append(nc.sync.dma_start(
                    ksh[0, :, 4 * w:4 * (w + 1), :], kst[:], cond=par0))
                k_writes.append(nc.sync.dma_start(
                    ksh[1, :, 4 * w:4 * (w + 1), :], kst[:], cond=par1))
            # pair barrier: peer's kT half is in shared HBM once this fires
            kb_in = dr_pool.tile([1, 64], F32, tag="kbi")
            kb_out = dr_pool.tile([2, 64], F32, tag="kbo")
            kbar = nc.gpsimd.collective_compute(
                "AllGather", mybir.AluOpType.bypass, replica_groups=PAIRS,
                ins=[kb_in.opt()], outs=[kb_out.opt()],
            )
            for wdma in k_writes:
                _add_dep_helper(kbar.ins, wdma.ins, sync=True,
                                reason="k barrier after shared-HBM writes")

            # ---- stage B: v own half [s0, e]; exchange per s-wave ----
            for w in range(2):
                grp = [(sT, ec) for sT in range(4 * w, 4 * w + 4)
                       for ec in range(2)]
                pss = wave_psums()
                for d in range(ND):
                    for g, (sT, ec) in enumerate(grp):
                        nc.tensor.matmul(
                            pss[g][:],
                            xk[d][:, sT * P:(sT + 1) * P],
                            wvq[d][:, ec * QB:(ec + 1) * QB],
                            start=(d == 0), stop=(d == ND - 1),
                        )
                vst = st_pool.tile([P, 4, D], BF16, tag="st",
                                   name=f"vst{w}")
                for g, (sT, ec) in enumerate(grp):
                    nc.vector.tensor_copy(
                        vst[:, sT - 4 * w, ec * QB:(ec + 1) * QB], pss[g][:])
                v_writes.append(nc.sync.dma_start(
                    vsh[0, :, 4 * w:4 * (w + 1), :], vst[:], cond=par0))
                v_writes.append(nc.sync.dma_start(
                    vsh[1, :, 4 * w:4 * (w + 1), :], vst[:], cond=par1))
            vb_in = dr_pool.tile([1, 64], F32, tag="vbi")
            vb_out = dr_pool.tile([2, 64], F32, tag="vbo")
            vbar = nc.gpsimd.collective_compute(
                "AllGather", mybir.AluOpType.bypass, replica_groups=PAIRS,
                ins=[vb_in.opt()], outs=[vb_out.opt()],
            )
            for wdma in v_writes:
                _add_dep_helper(vbar.ins, wdma.ins, sync=True,
                                reason="v barrier after shared-HBM writes")

            # ---- exchange read-backs (symmetric: both slots, no cond) ----
            for slot in range(2):
                rd = nc.sync.dma_start(
                    kT_big[:, :, slot * SQ:(slot + 1) * SQ], ksh[slot])
                _add_dep_helper(rd.ins, kbar.ins, sync=True,
                                reason="kT read after pair barrier")
            for slot in range(2):
                rd = nc.sync.dma_start(
                    v_big[:, 8 * slot:8 * (slot + 1), :], vsh[slot])
                _add_dep_helper(rd.ins, vbar.ins, sync=True,
                                reason="v read after pair barrier")

            # ---- stage C: qT[e, i] from own rows ----
            qT_t = [qT_pool.tile([P, SQ], BF16, tag="qT", name=f"qT{E}")
                    for E in range(NE)]
            for w in range(2):
                grp = [(E, qc) for E in range(4 * w, 4 * w + 4)
                       for qc in range(2)]
                pss = wave_psums()
                for d in range(ND):
                    for g, (E, qc) in enumerate(grp):
                        nc.tensor.matmul(
                            pss[g][:],
                            wvq[d][:, D + E * P:D + (E + 1) * P],
                            xk[d][:, qc * QB:(qc + 1) * QB],
                            start=(d == 0), stop=(d == ND - 1),
                        )
                for g, (E, qc) in enumerate(grp):
                    nc.vector.tensor_copy(
                        qT_t[E][:, qc * QB:(qc + 1) * QB], pss[g][:])

            # ---- stage D: attention per q block ----
            for j in range(NQB):
                sk_list = _sk_list(j)
                cross = _cross_list(j)
                # score order: interleave parity halves and put low-offset
                # (AV-gating) tiles first, so the exp/mask chain for each AV
                # u-group completes as early as possible
                sk_order = sorted(sk_list, key=lambda t: (_coff(j, t), t % 8))
                exp_pos = {t: i for i, t in enumerate(sk_order)}
                wtiles = {}
                for t in sk_order:
                    c = _coff(j, t)
                    w0 = c * P          # first live q column of this tile
                    ps = ps_pool.tile([P, QB], F32, tag="ps")
                    for E in range(NE):
                        nc.tensor.matmul(
                            ps[:, 0:QB - w0],
                            kT_big[:, E, t * P:(t + 1) * P],
                            qT_t[E][:, j * QB + w0:(j + 1) * QB],
                            start=(E == 0), stop=(E == NE - 1),
                        )
                    wt = we_pool.tile([P, QB], BF16, tag="we")
                    nc.scalar.activation(wt[:, w0:QB], ps[:, 0:QB - w0],
                                         AF.Exp, scale=float(SCALE))
                    if t in cross:
                        tt = cross.index(t)
                        nc.vector.tensor_mul(wt[:, w0:QB], wt[:, w0:QB],
                                             mask_big[:, j, tt, w0:QB])
                    wtiles[t] = wt

                for u in range(QB // P):
                    # accumulate in exp-completion order so the group's first
                    # matmuls never wait on late-scored tiles
                    ts_u = sorted(
                        (t for t in sk_list if _coff(j, t) <= u),
                        key=lambda t: exp_pos[t])
                    av0 = av_pool.tile([P, QB], F32, tag="av")
                    av1 = av_pool.tile([P, QB], F32, tag="av")
                    rs = rs_pool.tile([P, 1], F32, tag="rs")
                    n = len(ts_u)
                    for idx, t in enumerate(ts_u):
                        lhsT = wtiles[t][:, u * P:(u + 1) * P]
                        st, sp = idx == 0, idx == n - 1
                        nc.tensor.matmul(av0[:], lhsT, v_big[:, t, 0:QB],
                                         start=st, stop=sp)
                        nc.tensor.matmul(av1[:], lhsT, v_big[:, t, QB:D],
                                         start=st, stop=sp)
                        nc.tensor.matmul(rs[:], lhsT, ones_t[:, 0:1],
                                         start=st, stop=sp)
                    rcp = rc_pool.tile([P, 1], F32, tag="rcp")
                    nc.vector.reciprocal(rcp[:], rs[:])
                    ot = o_pool.tile([P, D], BF16, tag="o")
                    r0 = (j * (QB // P) + u) * P
                    nc.vector.tensor_scalar_mul(ot[:, 0:QB], av0[:], rcp[:])
                    nc.sync.dma_start(out[r0:r0 + P, 0:QB], ot[:, 0:QB])
                    nc.vector.tensor_scalar_mul(ot[:, QB:D], av1[:], rcp[:])
                    nc.sync.dma_start(out[r0:r0 + P, QB:D], ot[:, QB:D])

    nc.compile()
    return nc


def _prep_inputs(x, Wq, Wk, Wv):
    bf = ml_dtypes.bfloat16

    def tiled(a):     # [D, n] -> [ND, P, n]
        return a.reshape(ND, P, a.shape[1]).astype(bf)

    # weights are used as lhsT in natural [d, e] layout
    wq_b = tiled(Wq)
    wk_b = tiled(Wk)
    wv_b = tiled(Wv)
    wvq_b = np.ascontiguousarray(np.concatenate([wv_b, wq_b], axis=2))
    ones = np.ones((P, 8), bf)
    ks = np.arange(S)
    ii = np.arange(SQ)
    # global index of permuted key position (parity-0 rows, then parity-1)
    gk = np.where(ks < SQ, 2 * ks, 2 * (ks - SQ) + 1)
    in_maps = []
    for c in range(NCORES):
        b, p = c // 2, c % 2
        xoT = x[b, p::2].T                          # [D, SQ]
        xk_b = np.ascontiguousarray(
            np.concatenate([tiled(xoT), wk_b], axis=2))
        gq = 2 * ii + p
        maskd = np.zeros((NQB, 8, P, QB), np.float32)
        for j in range(NQB):
            for tt, t in enumerate(_cross_list(j)):
                gk_t = gk[t * P:(t + 1) * P]
                gq_j = gq[QB * j:QB * (j + 1)]
                maskd[j, tt] = (gk_t[:, None] <= gq_j[None, :]).astype(np.float32)
        # device layout [P, NQB, 8, QB] (partition-major, contiguous rows)
        mask_dev = np.ascontiguousarray(
            maskd.transpose(2, 0, 1, 3).astype(bf))
        in_maps.append({
            "xkt": xk_b, "wvqt": wvq_b,
            "maskd": mask_dev, "ones": ones,
        })
    return in_maps


def kernel(x, Wq, Wk, Wv):
    global LAST_RESULT
    x = np.asarray(x, np.float32)
    Wq = np.asarray(Wq, np.float32)
    Wk = np.asarray(Wk, np.float32)
    Wv = np.asarray(Wv, np.float32)

    if "nc" not in _cache:
        _cache["nc"] = _build()
    nc = _cache["nc"]

    in_maps = _prep_inputs(x, Wq, Wk, Wv)
    res = run_bass_kernel_spmd(nc, in_maps, list(range(NCORES)), trace=TRACE)
    LAST_RESULT = res

    out = np.empty((B, S, D), np.float32)
    for c in range(NCORES):
        b, p = c // 2, c % 2
        out[b, p::2, :] = res.results[c]["out"].astype(np.float32)
    return out
